# revision 41
# baseline (speedup 1.0000x reference)
"""Trainium2 Bass kernel for nn_Decoder (LSTM-style decoder with r/dt side path).

Reference math (per step t, teacher forcing):
    xs_t    = SOS one-hot (t=0) or input_seq[:, t-1]
    z       = xs_t @ w2h_w.T + w2h_b + hid @ h2h_w.T + h2h_b          (B, 4H)
    gi,gf,go = sigmoid(z[:, 0:H]), sigmoid(z[:, H:2H]), sigmoid(z[:, 2H:3H])
    chat    = tanh(z[:, 3H:4H])
    gr      = sigmoid(xs_t @ w2h_r_w.T + w2h_r_b + a*(hid @ h2h_r_w.T + h2h_r_b))
    dt      = gr * dt
    cell    = gf*cell + gi*chat + dt @ dc_w.T
    hid     = go * tanh(cell)
    logits  = hid @ out_w.T + out_b

Distribution: tensor-parallel over H across 8 cores (128 H-dims per core).
  - GEMM1 (xs @ w2h.T): each core computes its 512 gate rows for all 4800
    (t,b) columns; r-projection is column-split 600/core + one AllGather.
  - scan: per-core 640-dim gate slice, per-step 16KB AllGather of hidT chunks.
  - GEMM2 (logits): vocab-split 1000/core.
All layouts are transposed: feature dims on SBUF partitions, (t,b) on free.
"""

import functools

import numpy as np
import ml_dtypes

B = 64
T = 75
V = 8000
H = 1024
D = 128
ALPHA = 0.5
NCORE = 8
HC = H // NCORE          # 128: per-core hidden chunk
GS = 4 * HC              # 512: per-core gate rows
VS = V // NCORE          # 1000: per-core vocab slice
TB = T * B               # 4800
V_PAD = 8064             # 63 * 128
KV = V_PAD // 128        # 63 K-tiles for GEMM1
KH = H // 128            # 8 K-tiles for the scan / GEMM2
NCH = 200                # GEMM1 n-chunk columns (divides TB and TB/NCORE)
NCHUNKS = TB // NCH      # 24
RCH = (TB // NCORE) // NCH   # 5 r-chunks per core

BF16 = ml_dtypes.bfloat16
DEBUG_TAPS = False
# True = issue all GEMM1 units before the scan; False = interleave GEMM1
# units between scan steps so they fill the AllGather wait gaps (~0.5ms
# faster). Both validated on hardware at full size.
G1_PROLOGUE_ALL = False
NO_COLLECTIVES = False  # timing-bisect only: replaces AGs with local DMAs (WRONG results)
# Exchange hidden-state chunks with direct SBUF->SBUF remote DMA broadcasts
# (XOR slot addressing) instead of ncfw AllGather collectives (~15us each).
REMOTE_EXCHANGE = False


def _build_module(t_steps=T, v_pad=V_PAD, nch=NCH, vs=VS):
    import concourse.mybir as mybir
    import concourse.tile as tile
    from concourse import bacc

    dt_ = mybir.dt
    f32, bf16 = dt_.float32, dt_.bfloat16
    AF = mybir.ActivationFunctionType
    ALU = mybir.AluOpType

    kv = v_pad // 128
    tb = t_steps * B
    nchunks = tb // nch
    rch = (tb // NCORE) // nch
    RG = [list(range(NCORE))]
    # GEMM2 column split into <=500-wide pieces (PSUM bank limit)
    g2_splits = []
    col = 0
    while col < vs:
        w = min(500, vs - col)
        g2_splits.append((col, w))
        col += w

    nc = bacc.Bacc("TRN2", target_bir_lowering=False, num_devices=NCORE)

    # ---------------- I/O ----------------
    xs_ch = nc.dram_tensor("xs_ch", [nchunks, 128, kv, nch], bf16, kind="ExternalInput")
    xs_r = nc.dram_tensor("xs_r", [rch, 128, kv, nch], bf16, kind="ExternalInput")
    w1T = nc.dram_tensor("w1T", [v_pad, GS + D], bf16, kind="ExternalInput")
    wcatT = nc.dram_tensor("wcatT", [H, 5 * HC], bf16, kind="ExternalInput")
    dcT = nc.dram_tensor("dcT", [D, HC], bf16, kind="ExternalInput")
    owT = nc.dram_tensor("owT", [H, vs], bf16, kind="ExternalInput")
    obB = nc.dram_tensor("obB", [128, vs], f32, kind="ExternalInput")
    biasC = nc.dram_tensor("biasC", [128, 5], f32, kind="ExternalInput")
    identI = nc.dram_tensor("identI", [128, 128], bf16, kind="ExternalInput")
    hidT0 = nc.dram_tensor("hidT0", [H, B], bf16, kind="ExternalInput")
    cellT0 = nc.dram_tensor("cellT0", [HC, B], f32, kind="ExternalInput")
    dtT0 = nc.dram_tensor("dtT0", [D, B], f32, kind="ExternalInput")
    outc = nc.dram_tensor("outc", [tb, vs], f32, kind="ExternalOutput")
    if DEBUG_TAPS:
        dbg_pre = nc.dram_tensor("dbg_pre", [5 * HC, tb], bf16, kind="ExternalOutput")
        dbg_hid = nc.dram_tensor("dbg_hid", [t_steps, H, B], bf16, kind="ExternalOutput")

    if REMOTE_EXCHANGE:
        # hidden-state history for GEMM2, staged per step from SBUF
        hidst = nc.dram_tensor("hidst", [t_steps, 128, KH, B], bf16)
        recv_sem = nc.alloc_semaphore("rdma_recv")
        prep_sem = nc.alloc_semaphore("rdma_prep")
        lsem = nc.alloc_semaphore("rdma_local")
    else:
        # per-step AllGather buffers (must persist until GEMM2)
        agi = [nc.dram_tensor(f"agi{t}", [128, B], bf16) for t in range(t_steps)]
        ago = [
            nc.dram_tensor(f"ago{t}", [H, B], bf16, addr_space="Shared")
            for t in range(t_steps)
        ]
    agr_i = nc.dram_tensor("agr_i", [128, tb // NCORE], bf16)
    agr_o = nc.dram_tensor("agr_o", [H, tb // NCORE], bf16, addr_space="Shared")

    with tile.TileContext(nc) as tc:
        import contextlib

        with contextlib.ExitStack() as ctx:
            cpool = ctx.enter_context(tc.tile_pool(name="const", bufs=1))
            spool = ctx.enter_context(tc.tile_pool(name="state", bufs=1))
            wpool = ctx.enter_context(tc.tile_pool(name="work", bufs=3))
            hpool = ctx.enter_context(tc.tile_pool(name="hid", bufs=3))
            # ---- resident constants ----
            wcat_sb = cpool.tile([128, KH, 5 * HC], bf16)
            nc.sync.dma_start(wcat_sb[:], wcatT.ap().rearrange("(k p) m -> p k m", p=128))
            dc_sb = cpool.tile([128, HC], bf16)
            nc.sync.dma_start(dc_sb[:], dcT.ap())
            bias_sb = cpool.tile([128, 5], f32)
            nc.sync.dma_start(bias_sb[:], biasC.ap())
            id_sb = cpool.tile([128, 128], bf16)
            nc.sync.dma_start(id_sb[:], identI.ap())
            preG = cpool.tile([128, 4, tb], bf16)
            preR = cpool.tile([128, tb], bf16)

            # ---- state ----
            cell_sb = spool.tile([128, B], f32)
            nc.sync.dma_start(cell_sb[:], cellT0.ap())
            dt_sb = spool.tile([128, B], f32)
            nc.sync.dma_start(dt_sb[:], dtT0.ap())

            if REMOTE_EXCHANGE:
                hstA = spool.tile([128, KH, B], bf16, name="hstA")
                hstB = spool.tile([128, KH, B], bf16, name="hstB")
                hcur = hstA
                nc.sync.dma_start(
                    hcur[:], hidT0.ap().rearrange("(k p) n -> p k n", p=128)
                )
            else:
                hcur = hpool.tile([128, KH, B], bf16, tag="hstage")
                nc.sync.dma_start(
                    hcur[:], hidT0.ap().rearrange("(k p) n -> p k n", p=128)
                )

            with contextlib.ExitStack() as c1:
                g1pool = c1.enter_context(tc.tile_pool(name="g1", bufs=1))
                xpool = c1.enter_context(tc.tile_pool(name="xs", bufs=2))
                gpsum = c1.enter_context(
                    tc.tile_pool(name="gpsum", bufs=2, space="PSUM")
                )
                zpsum = c1.enter_context(
                    tc.tile_pool(name="zpsum", bufs=1, space="PSUM")
                )
                dpsum = c1.enter_context(
                    tc.tile_pool(name="dpsum", bufs=1, space="PSUM")
                )

                w1_sb = g1pool.tile([128, kv, GS + D], bf16)
                nc.sync.dma_start(
                    w1_sb[:], w1T.ap().rearrange("(k p) m -> p k m", p=128)
                )

                # ---- prologue: r-projection (column slice) + AllGather ----
                prr = g1pool.tile([128, tb // NCORE], bf16)
                for i in range(rch):
                    xt = xpool.tile([128, kv, nch], bf16, tag="xsch")
                    nc.sync.dma_start(xt[:], xs_r.ap()[i])
                    pg = gpsum.tile([128, nch], f32, tag="gps")
                    for k in range(kv):
                        nc.tensor.matmul(
                            pg[:],
                            w1_sb[:, k, GS : GS + D],
                            xt[:, k, :],
                            start=(k == 0),
                            stop=(k == kv - 1),
                        )
                    nc.vector.tensor_scalar_add(
                        prr[:, i * nch : (i + 1) * nch], pg[:], bias_sb[:, 0:1]
                    )
                nc.sync.dma_start(agr_i.ap(), prr[:])
                if NO_COLLECTIVES:
                    for j in range(NCORE):
                        nc.sync.dma_start(
                            agr_o.ap()[j * 128 : (j + 1) * 128], agr_i.ap()
                        )
                    for t in range(t_steps):
                        nc.sync.dma_start(ago[t].ap(), hidT0.ap())
                else:
                    nc.gpsimd.collective_compute(
                        "AllGather",
                        ALU.bypass,
                        replica_groups=RG,
                        ins=[agr_i.ap().opt()],
                        outs=[agr_o.ap().opt()],
                    )
                nc.sync.dma_start(
                    preR[:].rearrange("p (c f) -> p c f", c=NCORE),
                    agr_o.ap().rearrange("(c p) f -> p c f", p=128),
                )

                # ---- GEMM1 gate units, interleaved with the scan ----
                xs_tiles = {}

                def chunk_dma(ch):
                    if ch in xs_tiles or ch >= nchunks:
                        return
                    xt = xpool.tile([128, kv, nch], bf16, tag="xsch")
                    nc.sync.dma_start(xt[:], xs_ch.ap()[ch])
                    xs_tiles[ch] = xt

                state = {"issued": 0}

                def issue_units(target):
                    while state["issued"] < min(target, 4 * nchunks):
                        u = state["issued"]
                        ch, g = u // 4, u % 4
                        if g == 0:
                            chunk_dma(ch)
                            chunk_dma(ch + 1)
                        pg = gpsum.tile([128, nch], f32, tag="gps")
                        for k in range(kv):
                            nc.tensor.matmul(
                                pg[:],
                                w1_sb[:, k, g * HC : (g + 1) * HC],
                                xs_tiles[ch][:, k, :],
                                start=(k == 0),
                                stop=(k == kv - 1),
                            )
                        nc.vector.tensor_scalar_add(
                            preG[:, g, ch * nch : (ch + 1) * nch],
                            pg[:],
                            bias_sb[:, 1 + g : 2 + g],
                        )
                        state["issued"] += 1
                        if state["issued"] % 4 == 0:
                            xs_tiles.pop(state["issued"] // 4 - 1, None)

                def g1_target(t):
                    if G1_PROLOGUE_ALL:
                        return 4 * nchunks
                    deadline = 4 * (((t + 2) * B) // nch + 1)
                    den = max(1, t_steps - 3)
                    pace = (4 * nchunks * (t + 1) + den - 1) // den
                    return min(4 * nchunks, max(deadline, pace))

                issue_units(g1_target(0))

                # ---- the scan ----
                for t in range(t_steps):
                    # one PSUM tile (= one bank) per gate region: start=True
                    # clears has_written for the WHOLE bank, so accumulation
                    # groups must not share a bank.
                    pz = [
                        zpsum.tile([128, B], f32, tag=f"pz{m}", name=f"pz{m}_{t}")
                        for m in range(5)
                    ]
                    # inject pre-projections (identity matmul, one LDW)
                    for m in range(5):
                        pre_ap = (
                            preR[:, t * B : (t + 1) * B]
                            if m == 0
                            else preG[:, m - 1, t * B : (t + 1) * B]
                        )
                        nc.tensor.matmul(
                            pz[m][:], id_sb[:], pre_ap, start=True, stop=False
                        )
                    # recurrent matmuls
                    for m in range(5):
                        for k in range(KH):
                            nc.tensor.matmul(
                                pz[m][:],
                                wcat_sb[:, k, m * HC : (m + 1) * HC],
                                hcur[:, k, :],
                                start=False,
                                stop=(k == KH - 1),
                            )
                    # activations: [r | gi | gf | go] sigmoid, [chat] tanh
                    sg = wpool.tile([128, 4 * B], f32, tag="sg")
                    for m in range(4):
                        nc.scalar.activation(
                            sg[:, m * B : (m + 1) * B], pz[m][:], AF.Sigmoid
                        )
                    th = wpool.tile([128, B], f32, tag="th")
                    nc.scalar.activation(th[:], pz[4][:], AF.Tanh)
                    # dt = gr * dt ; dc = dcT.T @ dt
                    nc.vector.tensor_mul(dt_sb[:], sg[:, 0:B], dt_sb[:])
                    dtb = wpool.tile([128, B], bf16, tag="dtb")
                    nc.vector.tensor_copy(dtb[:], dt_sb[:])
                    pdc = dpsum.tile([128, B], f32, tag="pdc")
                    nc.tensor.matmul(pdc[:], dc_sb[:], dtb[:], start=True, stop=True)
                    # cell = gf*cell + gi*chat + dc
                    tmp = wpool.tile([128, B], f32, tag="tmp")
                    nc.vector.tensor_mul(tmp[:], sg[:, B : 2 * B], th[:])
                    nc.vector.tensor_mul(cell_sb[:], sg[:, 2 * B : 3 * B], cell_sb[:])
                    nc.vector.tensor_add(cell_sb[:], cell_sb[:], tmp[:])
                    nc.vector.tensor_add(cell_sb[:], cell_sb[:], pdc[:])
                    # hid = go * tanh(cell)
                    thc = wpool.tile([128, B], f32, tag="thc")
                    nc.scalar.activation(thc[:], cell_sb[:], AF.Tanh)
                    hch = wpool.tile([128, B], bf16, tag="hch")
                    nc.vector.tensor_mul(hch[:], sg[:, 3 * B : 4 * B], thc[:])
                    # issue GEMM1 filler work BEFORE the exchange so it can
                    # run on the PE while the exchange is in flight
                    if t + 1 < t_steps:
                        issue_units(g1_target(t + 1))
                    # exchange hidden chunks
                    if REMOTE_EXCHANGE:
                        hnx = hstB if t % 2 == 0 else hstA
                        with tc.tile_critical():
                            g = nc.gpsimd
                            if t >= 1:
                                g.wait_ge(lsem, 128 * t)
                            for k in range(NCORE):
                                g.remote_dma_broadcast(
                                    out_ap=hnx[:, k, :],
                                    in_ap=hch[:],
                                    remote_sem=recv_sem,
                                    local_sem=lsem,
                                    rdests=[
                                        (0, j) if j == k else None
                                        for j in range(NCORE)
                                    ],
                                ).then_inc(prep_sem, 1)
                            g.wait_ge(prep_sem, NCORE * (t + 1))
                            g.trigger_dma(count=NCORE)
                            # arrival fence: sync engine waits for all 16
                            # lane-halves, then self-copies the staging tile
                            # so Tile's dependency tracking gates all its
                            # readers on actual data arrival.
                            nc.sync.wait_ge(recv_sem, 2 * NCORE * (t + 1))
                            nc.sync.dma_start(hnx[:], hnx[:])
                        nc.sync.dma_start(hidst.ap()[t], hnx[:])
                        hcur = hnx
                    else:
                        nc.sync.dma_start(agi[t].ap(), hch[:])
                        if NO_COLLECTIVES:
                            nc.sync.dma_start(ago[t].ap()[0:128], agi[t].ap())
                        else:
                            nc.gpsimd.collective_compute(
                                "AllGather",
                                ALU.bypass,
                                replica_groups=RG,
                                ins=[agi[t].ap().opt()],
                                outs=[ago[t].ap().opt()],
                            )
                        if t + 1 < t_steps:
                            hcur = hpool.tile([128, KH, B], bf16, tag="hstage")
                            nc.sync.dma_start(
                                hcur[:],
                                ago[t].ap().rearrange("(k p) n -> p k n", p=128),
                            )

            if DEBUG_TAPS:
                for t in range(t_steps):
                    dcp = wpool.tile([128, KH, B], bf16, tag="dcp")
                    nc.sync.dma_start(
                        dcp[:], ago[t].ap().rearrange("(k p) n -> p k n", p=128)
                    )
                    nc.sync.dma_start(
                        dbg_hid.ap()[t].rearrange("(k p) n -> p k n", p=128), dcp[:]
                    )
                nc.sync.dma_start(dbg_pre.ap()[0:HC], preR[:])
                for g in range(4):
                    nc.sync.dma_start(
                        dbg_pre.ap()[(1 + g) * HC : (2 + g) * HC], preG[:, g, :]
                    )

            # ---- GEMM2: logits = hidT.T @ owT + ob ----
            with contextlib.ExitStack() as c2:
                g2pool = c2.enter_context(tc.tile_pool(name="g2", bufs=3))
                opsum = c2.enter_context(
                    tc.tile_pool(name="opsum", bufs=2, space="PSUM")
                )
                ow_sb = g2pool.tile([128, KH, vs], bf16, tag="owt", bufs=1)
                nc.sync.dma_start(
                    ow_sb[:], owT.ap().rearrange("(k p) n -> p k n", p=128)
                )
                ob_sb = g2pool.tile([128, vs], f32, tag="obb", bufs=1)
                nc.sync.dma_start(ob_sb[:], obB.ap())

                n_rb = (tb + 127) // 128
                for rb in range(n_rb):
                    rows = min(128, tb - rb * 128)
                    lh = g2pool.tile([128, KH, 128], bf16, tag="g2lh")
                    if REMOTE_EXCHANGE:
                        nc.sync.dma_start(lh[:, :, 0:B], hidst.ap()[2 * rb])
                        if rows > B:
                            nc.sync.dma_start(lh[:, :, B:128], hidst.ap()[2 * rb + 1])
                    else:
                        nc.sync.dma_start(
                            lh[:, :, 0:B],
                            ago[2 * rb].ap().rearrange("(k p) n -> p k n", p=128),
                        )
                        if rows > B:
                            nc.sync.dma_start(
                                lh[:, :, B:128],
                                ago[2 * rb + 1]
                                .ap()
                                .rearrange("(k p) n -> p k n", p=128),
                            )
                    # one PSUM tile per split: a matmul output must not cross
                    # a 2KB bank boundary
                    osb = g2pool.tile([128, vs], f32, tag="osb")
                    for j, (c0, w) in enumerate(g2_splits):
                        po = opsum.tile(
                            [128, w], f32, tag=f"po{j}", name=f"po{j}_{rb}"
                        )
                        for k in range(KH):
                            nc.tensor.matmul(
                                po[:rows],
                                lh[:, k, 0:rows],
                                ow_sb[:, k, c0 : c0 + w],
                                start=(k == 0),
                                stop=(k == KH - 1),
                            )
                        nc.vector.tensor_add(
                            osb[:rows, c0 : c0 + w],
                            po[:rows],
                            ob_sb[:rows, c0 : c0 + w],
                        )
                    nc.sync.dma_start(
                        outc.ap()[rb * 128 : rb * 128 + rows, :], osb[:rows]
                    )

    nc.finalize()
    return nc


@functools.lru_cache(maxsize=2)
def _cached_module(t_steps, v_pad, nch, vs):
    return _build_module(t_steps, v_pad, nch, vs)


def _prep_inputs(
    input_seq, last_hidden, last_dt, w2h_w, w2h_b, h2h_w, h2h_b,
    w2h_r_w, w2h_r_b, h2h_r_w, h2h_r_b, dc_w, out_w, out_b,
):
    """Host-side sharding/layout. Returns per-core input dicts."""
    b, t_steps, v = input_seq.shape
    h = last_hidden.shape[1]
    d = last_dt.shape[1]
    tb = t_steps * b
    v_pad = ((v + 127) // 128) * 128
    kv = v_pad // 128
    # choose n-chunk: must divide tb and tb // NCORE
    nch = NCH if (tb % NCH == 0 and (tb // NCORE) % NCH == 0) else (tb // NCORE)
    while tb % nch or (tb // NCORE) % nch:
        nch //= 2
    nchunks = tb // nch
    rch = (tb // NCORE) // nch
    vs = v // NCORE

    # xsT: (v_pad, tb) with col t*B+b = SOS (t=0) or input_seq[b, t-1]
    xsT = np.zeros((v_pad, tb), np.float32)
    xsT[0, 0:b] = 1.0
    xsT[:v, b:] = input_seq[:, : t_steps - 1, :].transpose(2, 1, 0).reshape(v, tb - b)
    xsT = xsT.astype(BF16)
    # chunked layout (nchunks, 128, kv, nch)
    xs_ch = np.ascontiguousarray(
        xsT.reshape(kv, 128, nchunks, nch).transpose(2, 1, 0, 3)
    )

    gate_bias = (w2h_b + h2h_b).astype(np.float32)
    r_bias = (w2h_r_b + ALPHA * h2h_r_b).astype(np.float32)

    ident = np.eye(128, dtype=BF16)
    hidT0 = np.ascontiguousarray(last_hidden.T).astype(BF16)
    dtT0 = np.ascontiguousarray(last_dt.T).astype(np.float32)
    cellT0_full = np.ascontiguousarray(last_hidden.T).astype(np.float32)

    wrT = np.zeros((v_pad, d), BF16)
    wrT[:v] = w2h_r_w.T.astype(BF16)
    wcat_r = (ALPHA * h2h_r_w).T.astype(BF16)  # (h, d)

    in_maps = []
    for c in range(NCORE):
        idx = np.concatenate(
            [np.arange(g * h + c * HC, g * h + (c + 1) * HC) for g in range(4)]
        )
        # row permutation of the H axis: with REMOTE_EXCHANGE, staging slot j
        # on core c holds H-chunk (c XOR j), so per-core H-contracted weights
        # are supplied with their K-tiles in that order.
        if REMOTE_EXCHANGE:
            hperm = np.concatenate(
                [np.arange((c ^ j) * HC, (c ^ j) * HC + HC) for j in range(NCORE)]
            )
        else:
            hperm = np.arange(h)
        w1 = np.zeros((v_pad, GS + d), BF16)
        w1[:v, :GS] = w2h_w[idx].T.astype(BF16)
        w1[:, GS:] = wrT
        wcat = np.concatenate([wcat_r, h2h_w[idx].T.astype(BF16)], axis=1)[hperm]
        biasC = np.zeros((128, 5), np.float32)
        biasC[:, 0] = r_bias
        for g in range(4):
            biasC[:, 1 + g] = gate_bias[g * h + c * HC : g * h + (c + 1) * HC]
        in_maps.append(
            {
                "xs_ch": xs_ch,
                "xs_r": xs_ch[c * rch : (c + 1) * rch],
                "w1T": w1,
                "wcatT": np.ascontiguousarray(wcat),
                "dcT": np.ascontiguousarray(dc_w[c * HC : (c + 1) * HC, :].T).astype(
                    BF16
                ),
                "owT": np.ascontiguousarray(
                    out_w[c * vs : (c + 1) * vs, :].T[hperm]
                ).astype(BF16),
                "obB": np.ascontiguousarray(
                    np.broadcast_to(
                        out_b[c * vs : (c + 1) * vs].astype(np.float32), (128, vs)
                    )
                ),
                "biasC": biasC,
                "identI": ident,
                "hidT0": np.ascontiguousarray(hidT0[hperm]),
                "cellT0": np.ascontiguousarray(cellT0_full[c * HC : (c + 1) * HC]),
                "dtT0": dtT0,
            }
        )
    return in_maps, nch, v_pad, vs


def kernel(**inputs):
    from concourse.bass_utils import run_bass_kernel_spmd

    input_seq = np.asarray(inputs["input_seq"], np.float32)
    b, t_steps, v = input_seq.shape
    args = {
        k: np.asarray(inputs[k], np.float32)
        for k in (
            "last_hidden", "last_dt", "w2h_w", "w2h_b", "h2h_w", "h2h_b",
            "w2h_r_w", "w2h_r_b", "h2h_r_w", "h2h_r_b", "dc_w", "out_w", "out_b",
        )
    }
    in_maps, nch, v_pad, vs = _prep_inputs(input_seq, **{
        "last_hidden": args["last_hidden"], "last_dt": args["last_dt"],
        "w2h_w": args["w2h_w"], "w2h_b": args["w2h_b"],
        "h2h_w": args["h2h_w"], "h2h_b": args["h2h_b"],
        "w2h_r_w": args["w2h_r_w"], "w2h_r_b": args["w2h_r_b"],
        "h2h_r_w": args["h2h_r_w"], "h2h_r_b": args["h2h_r_b"],
        "dc_w": args["dc_w"], "out_w": args["out_w"], "out_b": args["out_b"],
    })
    nc = _cached_module(t_steps, v_pad, nch, vs)
    res = run_bass_kernel_spmd(nc, in_maps, core_ids=list(range(NCORE)))
    stack = np.stack([res.results[c]["outc"] for c in range(NCORE)])  # (8, tb, vs)
    out = (
        stack.reshape(NCORE, t_steps, b, vs)
        .transpose(2, 1, 0, 3)
        .reshape(b, t_steps, NCORE * vs)
    )
    return np.ascontiguousarray(out)
